# revision 61
# baseline (speedup 1.0000x reference)
"""Trainium2 Bass kernel for nn_AttentionADCell.

Reference math (per batch row b, B=4096, D_IN=512, D_H=256, D_G=128, H=8):
  h  = LayerNorm(relu(x @ Wh + bh)) * ln_scale + ln_bias          (B, 256)
  z1 = (x @ Wz1).reshape(B, 8, 256)
  z2 = (x @ Wz2).reshape(B, 8, 128)
  w  = einsum('bhg,bhd->bgd', z2, z1)                              (B, 128, 256)
  w  = tanh((w - mean_d(w)) / (std_d(w) + 1e-6))
  y  = einsum('bgd,bd->bg', w, h)                                  (B, 128)
Returns (h, y).

Strategy: pure data parallel over batch across 8 NeuronCores (512 rows each).
Per core, rows are processed in 4 blocks of 128. Key ideas:
  - Standardization is folded into the einsum as a 9th "head":
      w' = (w - mu)*isd  ==  sum_h z1aug[h,d] * z2aug[h,g]
    with z1aug = [z1; ones], z2aug = [z2*isd; -mu*isd].  The einsum is done
    TRANSPOSED (d on partitions) so the final y contraction over d runs on
    the tensor engine.
  - mu and var are computed algebraically from small reductions:
      sum_d w = sum_h z2[h,g]*s1[h],  s1 = row-sums of z1
      sum_d w^2 = sum_{h,h'} z2[h]z2[h'] M1[h,h'],  M1 = z1 z1^T (8x8 Gram)
    M1 via 36 fused scalar_tensor_tensor+accum ops on the vector engine.
  - All einsum matmuls are K=9 at partition base 0 (tile_position packing is
    broken in this stack: zero in-repo usage, ignored by CoreSim, and any
    nonzero row position fails at runtime on HW).  y accumulates in a single
    PSUM bank in (g, row) orientation and is transposed once at the end.
  - fp16 for einsum operands / tanh output (fp32 accumulation in PSUM);
    the h path stays fp32 end-to-end.
"""

import os
import sys
for _p in ("/opt/trn_rl_repo", "/opt/pypackages"):
    if _p not in sys.path:
        sys.path.append(_p)

TP_ROW = os.environ.get("KT_NO_TPROW", "0") != "1"   # row-group packing
TP_COL = os.environ.get("KT_NO_TPCOL", "0") != "1"   # col-group packing
USE_DMAT = os.environ.get("KT_NO_DMAT", "0") != "1"  # hT via DMA-transpose
PHASE = os.environ.get("KT_PHASE", "full")  # h | z | noy | full

from contextlib import ExitStack

import numpy as np

import concourse.bass as bass
import concourse.bacc as bacc
import concourse.mybir as mybir
import concourse.tile as tile
from concourse.bass_utils import run_bass_kernel_spmd
from concourse.masks import make_identity

F32 = mybir.dt.float32
F32R = mybir.dt.float32r
F16 = mybir.dt.float16
AF = mybir.ActivationFunctionType
OP = mybir.AluOpType

D_IN, D_H, D_G, H, B = 512, 256, 128, 8, 4096
EPS_LN = 1e-6
EPS_STD = 1e-6
NCORES = 8
BLOC = B // NCORES          # rows per core = 512
NBLK = BLOC // 128          # 4 blocks of 128 rows

PAIRS = [(h, h) for h in range(H)] + [
    (h1, h2) for h1 in range(H) for h2 in range(h1 + 1, H)
]  # 8 diag + 28 off-diag = 36


def build(nc: bass.Bass, tc: tile.TileContext):
    x = nc.dram_tensor("x", [BLOC, D_IN], F32, kind="ExternalInput").ap()
    Wh = nc.dram_tensor("Wh", [D_IN, D_H], F32, kind="ExternalInput").ap()
    bh = nc.dram_tensor("bh", [D_H], F32, kind="ExternalInput").ap()
    Wz1 = nc.dram_tensor("Wz1", [D_IN, H * D_H], F32, kind="ExternalInput").ap()
    Wz2 = nc.dram_tensor("Wz2", [D_IN, H * D_G], F32, kind="ExternalInput").ap()
    ln_s = nc.dram_tensor("ln_scale", [D_H], F32, kind="ExternalInput").ap()
    ln_b = nc.dram_tensor("ln_bias", [D_H], F32, kind="ExternalInput").ap()
    h_out = nc.dram_tensor("h_out", [BLOC, D_H], F32, kind="ExternalOutput").ap()
    y_out = nc.dram_tensor("y_out", [BLOC, D_G], F32, kind="ExternalOutput").ap()

    with ExitStack() as ctx:
        _body(ctx, nc, tc, x, Wh, bh, Wz1, Wz2, ln_s, ln_b, h_out, y_out)


def _body(ctx, nc, tc, x, Wh, bh, Wz1, Wz2, ln_s, ln_b, h_out, y_out):
    consts = ctx.enter_context(tc.tile_pool(name="consts", bufs=1))
    sb = ctx.enter_context(tc.tile_pool(name="sb", bufs=2))
    strips = ctx.enter_context(tc.tile_pool(name="strips", bufs=2))
    stmp = ctx.enter_context(tc.tile_pool(name="stmp", bufs=1))
    small = ctx.enter_context(tc.tile_pool(name="small", bufs=2))
    scr = ctx.enter_context(tc.tile_pool(name="scr", bufs=4))
    twp = ctx.enter_context(tc.tile_pool(name="twp", bufs=2))
    psA = ctx.enter_context(tc.tile_pool(name="psA", bufs=1, space="PSUM"))
    psX = ctx.enter_context(tc.tile_pool(name="psX", bufs=1, space="PSUM"))
    psZ = ctx.enter_context(tc.tile_pool(name="psZ", bufs=1, space="PSUM"))
    psW = ctx.enter_context(tc.tile_pool(name="psW", bufs=1, space="PSUM"))
    psY = ctx.enter_context(tc.tile_pool(name="psY", bufs=1, space="PSUM"))

    # ---- constants ----
    ident32 = consts.tile([128, 128], F32)
    make_identity(nc, ident32)
    ones1_f32 = consts.tile([1, 128], F32)
    nc.vector.memset(ones1_f32, 1.0)
    onescol = consts.tile([128, 1], F32)
    nc.vector.memset(onescol, 1.0)
    zrow512 = consts.tile([1, 512], F32)
    nc.vector.memset(zrow512, 0.0)
    ones_h16 = consts.tile([1, 32 * D_H], F16)
    nc.vector.memset(ones_h16, 1.0)
    eps_ln = consts.tile([128, 1], F32)
    nc.vector.memset(eps_ln, EPS_LN)

    Wh_sb = consts.tile([128, 4, D_H], F32)
    nc.sync.dma_start(out=Wh_sb, in_=Wh.rearrange("(c p) n -> p c n", p=128))
    Wz1_sb = consts.tile([128, 4, H * D_H], F16)
    nc.gpsimd.dma_start(out=Wz1_sb, in_=Wz1.rearrange("(c p) n -> p c n", p=128))
    Wz2_sb = consts.tile([128, 4, H * D_G], F16)
    nc.gpsimd.dma_start(out=Wz2_sb, in_=Wz2.rearrange("(c p) n -> p c n", p=128))
    bh_sb = consts.tile([1, D_H], F32)
    nc.sync.dma_start(out=bh_sb, in_=bh.rearrange("(o n) -> o n", o=1))
    lns_sb = consts.tile([1, D_H], F32)
    nc.sync.dma_start(out=lns_sb, in_=ln_s.rearrange("(o n) -> o n", o=1))
    lnb_sb = consts.tile([1, D_H], F32)
    nc.sync.dma_start(out=lnb_sb, in_=ln_b.rearrange("(o n) -> o n", o=1))

    # broadcast ln scale/bias to (128, 256) via K=1 matmul
    lnsc_bc = consts.tile([128, D_H], F32)
    lnbi_bc = consts.tile([128, D_H], F32)
    bc_ps = psX.tile([128, D_H], F32, tag="psX")
    nc.tensor.matmul(bc_ps, lhsT=ones1_f32, rhs=lns_sb, start=True, stop=True)
    nc.scalar.copy(lnsc_bc, bc_ps)
    bc_ps2 = psX.tile([128, D_H], F32, tag="psX")
    nc.tensor.matmul(bc_ps2, lhsT=ones1_f32, rhs=lnb_sb, start=True, stop=True)
    nc.scalar.copy(lnbi_bc, bc_ps2)
    # dummy transpose: makes the PE observe the GPSIMD identity build once,
    # so per-block transposes stay within the 1-wait limit
    dmy_ps = psX.tile([128, 128], F32, tag="psX")
    nc.tensor.transpose(dmy_ps, ident32, ident32)

    # wt: 4 PSUM banks, two 2-bank halves used alternately by einsum groups
    wt = psW.tile([128, 16, 128], F32)
    # y accumulator: one bank, col = permuted row index for the whole core
    y_ps = psY.tile([128, BLOC], F32)
    nc.tensor.matmul(y_ps, lhsT=ones1_f32, rhs=zrow512, start=True,
                     stop=False, skip_group_check=True)

    for blk in range(NBLK):
        r0 = blk * 128

        # ---- P1: load x (rows PERMUTED), transpose on PE ----
        # SBUF partition p = 32*s + j holds batch row r0 + 4*j + s: the strip
        # shuffle then reads contiguous partition ranges; output DMAs
        # un-permute.
        x_sb = sb.tile([128, D_IN], F32, tag="x")
        nc.sync.dma_start(
            out=x_sb, in_=x[r0 : r0 + 128, :].rearrange("(j s) d -> s j d", s=4)
        )
        # column-sum matmul: makes the PE observe the x DMA so the
        # following transpose instructions need no DMA wait (1-wait limit)
        csum_ps = psZ.tile([128, 512], F32, tag="psZ")
        nc.tensor.matmul(
            csum_ps[0:1, :], lhsT=onescol, rhs=x_sb, start=True, stop=True
        )
        xt_ps = psX.tile([128, D_IN], F32, tag="psX")
        for c in range(4):
            nc.tensor.transpose(
                xt_ps[:, 128 * c : 128 * (c + 1)],
                x_sb[:, 128 * c : 128 * (c + 1)],
                ident32,
            )
        xt_sb = sb.tile([128, 4, 128], F32, tag="xt")
        nc.scalar.copy(xt_sb, xt_ps.rearrange("p (c b) -> p c b", c=4))
        xt16 = sb.tile([128, 4, 128], F16, tag="xt16")
        nc.scalar.copy(xt16, xt_ps.rearrange("p (c b) -> p c b", c=4))

        # ---- P2: h path ----
        h_ps = psA.tile([128, D_H], F32, tag="psA")
        for c in range(4):
            nc.tensor.matmul(
                h_ps, lhsT=xt_sb[:, c, :], rhs=Wh_sb[:, c, :],
                start=(c == 0), stop=False,
            )
        nc.tensor.matmul(h_ps, lhsT=ones1_f32, rhs=bh_sb, start=False, stop=True)
        hr = sb.tile([128, D_H], F32, tag="hr")
        nc.scalar.activation(hr, h_ps, AF.Relu)
        st6 = small.tile([128, 6], F32, tag="st6")
        nc.vector.bn_stats(st6, hr)
        agg = small.tile([128, 2], F32, tag="agg")
        nc.vector.bn_aggr(agg, st6)
        sdt = small.tile([128, 1], F32, tag="sdt")
        nc.scalar.activation(sdt, agg[:, 1:2], AF.Sqrt, bias=eps_ln)
        rstd = small.tile([128, 1], F32, tag="rstd")
        nc.vector.reciprocal(rstd, sdt)
        nmr = small.tile([128, 1], F32, tag="nmr")
        nc.vector.scalar_tensor_tensor(
            out=nmr, in0=agg[:, 0:1], scalar=-1.0, in1=rstd,
            op0=OP.mult, op1=OP.mult,
        )
        hh = sb.tile([128, D_H], F32, tag="hh")
        nc.scalar.activation(hh, hr, AF.Identity, bias=nmr, scale=rstd)
        ht1 = sb.tile([128, D_H], F32, tag="ht1")
        nc.vector.tensor_mul(ht1, hh, lnsc_bc)
        h_sb = sb.tile([128, D_H], F32, tag="h")
        nc.vector.tensor_add(h_sb, ht1, lnbi_bc)
        nc.sync.dma_start(
            out=h_out[r0 : r0 + 128, :].rearrange("(j s) d -> s j d", s=4),
            in_=h_sb,
        )
        hT16 = sb.tile([128, 2, 128], F16, tag="hT")
        if USE_DMAT:
            h16 = sb.tile([128, D_H], F16, tag="h16")
            nc.scalar.copy(h16, h_sb)
            for t in range(2):
                nc.sync.dma_start_transpose(
                    out=hT16[:, t, :], in_=h16[:, 128 * t : 128 * (t + 1)]
                )
        else:
            ht_ps = psA.tile([128, D_H], F32, tag="psA")
            for t in range(2):
                nc.tensor.transpose(
                    ht_ps[:, 128 * t : 128 * (t + 1)],
                    h_sb[:, 128 * t : 128 * (t + 1)],
                    ident32,
                )
            nc.scalar.copy(hT16, ht_ps.rearrange("p (t b) -> p t b", t=2))

        if PHASE == "h":
            y_z = sb.tile([128, D_G], F32, tag="yz")
            nc.vector.memset(y_z, 0.0)
            nc.sync.dma_start(
                out=y_out[r0 : r0 + 128, :].rearrange("(j s) g -> s j g", s=4),
                in_=y_z,
            )
            continue

        # ---- P3: z projections (fp16), cast to fp16 ----
        z1_16 = sb.tile([128, H, D_H], F16, tag="z1b")
        z2_16 = sb.tile([128, H, D_G], F16, tag="z2b")
        for n in range(4):
            z_ps = psZ.tile([128, 512], F32, tag="psZ")
            for c in range(4):
                nc.tensor.matmul(
                    z_ps,
                    lhsT=xt16[:, c, :],
                    rhs=Wz1_sb[:, c, 512 * n : 512 * (n + 1)],
                    start=(c == 0), stop=(c == 3),
                )
            nc.vector.tensor_copy(
                z1_16[:, 2 * n : 2 * (n + 1), :],
                z_ps.rearrange("p (h d) -> p h d", h=2),
            )
        for n in range(2):
            z_ps = psZ.tile([128, 512], F32, tag="psZ")
            for c in range(4):
                nc.tensor.matmul(
                    z_ps,
                    lhsT=xt16[:, c, :],
                    rhs=Wz2_sb[:, c, 512 * n : 512 * (n + 1)],
                    start=(c == 0), stop=(c == 3),
                )
            nc.vector.tensor_copy(
                z2_16[:, 4 * n : 4 * (n + 1), :],
                z_ps.rearrange("p (h g) -> p h g", h=4),
            )
        s1 = small.tile([128, H], F32, tag="s1")
        nc.vector.tensor_reduce(s1, z1_16, axis=mybir.AxisListType.X, op=OP.add)

        if PHASE == "z3":
            y_z = sb.tile([128, D_G], F32, tag="yz")
            nc.vector.memset(y_z, 0.0)
            nc.sync.dma_start(
                out=y_out[r0 : r0 + 128, :].rearrange("(j s) g -> s j g", s=4),
                in_=y_z,
            )
            continue

        # ---- P4: stats (M1 Gram, mu, q, isd) ----
        M1 = small.tile([128, 36], F32, tag="M1")
        for idx, (h1, h2) in enumerate(PAIRS):
            scrt = scr.tile([128, D_H], F16, tag="scr")
            nc.vector.scalar_tensor_tensor(
                out=scrt, in0=z1_16[:, h1, :], scalar=1.0,
                in1=z1_16[:, h2, :], op0=OP.mult, op1=OP.mult,
                accum_out=M1[:, idx : idx + 1],
            )
        M1d = small.tile([128, 28], F32, tag="M1d")
        nc.vector.tensor_scalar_mul(M1d, M1[:, 8:36], 2.0)

        if PHASE == "m1":
            y_z = sb.tile([128, D_G], F32, tag="yz")
            nc.vector.memset(y_z, 0.0)
            nc.vector.tensor_scalar_mul(y_z[:, 0:1], M1[:, 0:1], 0.0)
            nc.vector.tensor_scalar_mul(y_z[:, 1:2], M1d[:, 0:1], 0.0)
            nc.sync.dma_start(
                out=y_out[r0 : r0 + 128, :].rearrange("(j s) g -> s j g", s=4),
                in_=y_z,
            )
            continue

        S16 = small.tile([128, D_G], F16, tag="S16")
        nc.vector.tensor_scalar(
            out=S16, in0=z2_16[:, 0, :], scalar1=s1[:, 0:1], scalar2=None,
            op0=OP.mult,
        )
        for h in range(1, H):
            nc.vector.scalar_tensor_tensor(
                out=S16, in0=z2_16[:, h, :], scalar=s1[:, h : h + 1], in1=S16,
                op0=OP.mult, op1=OP.add,
            )
        Q16 = small.tile([128, D_G], F16, tag="Q16")
        for idx, (h1, h2) in enumerate(PAIRS):
            prod = scr.tile([128, D_G], F16, tag="scr2")
            nc.vector.tensor_mul(prod, z2_16[:, h1, :], z2_16[:, h2, :])
            m1col = (
                M1[:, idx : idx + 1] if idx < 8 else M1d[:, idx - 8 : idx - 7]
            )
            if idx == 0:
                nc.vector.tensor_scalar(
                    out=Q16, in0=prod, scalar1=m1col, scalar2=None, op0=OP.mult
                )
            else:
                nc.vector.scalar_tensor_tensor(
                    out=Q16, in0=prod, scalar=m1col, in1=Q16,
                    op0=OP.mult, op1=OP.add,
                )

        if PHASE == "q":
            y_z = sb.tile([128, D_G], F32, tag="yz")
            nc.vector.tensor_scalar_mul(y_z, Q16, 0.0)
            nc.sync.dma_start(
                out=y_out[r0 : r0 + 128, :].rearrange("(j s) g -> s j g", s=4),
                in_=y_z,
            )
            continue

        mean = small.tile([128, D_G], F32, tag="mean")
        nc.vector.tensor_scalar_mul(mean, S16, 1.0 / D_H)
        msq = small.tile([128, D_G], F32, tag="msq")
        nc.vector.tensor_mul(msq, mean, mean)
        vt = small.tile([128, D_G], F32, tag="vt")
        nc.vector.tensor_scalar_mul(vt, Q16, 1.0 / D_H)
        nc.vector.tensor_sub(vt, vt, msq)
        nc.vector.tensor_scalar_max(vt, vt, 0.0)
        sd = small.tile([128, D_G], F32, tag="sd")
        nc.scalar.activation(sd, vt, AF.Sqrt)
        nc.vector.tensor_scalar_add(sd, sd, EPS_STD)
        isd = small.tile([128, D_G], F32, tag="isd")
        nc.vector.reciprocal(isd, sd)
        isd16 = small.tile([128, D_G], F16, tag="isd16")
        nc.vector.tensor_copy(isd16, isd)
        m2_16 = small.tile([128, D_G], F16, tag="m2")
        nc.vector.scalar_tensor_tensor(
            out=m2_16, in0=mean, scalar=-1.0, in1=isd, op0=OP.mult, op1=OP.mult
        )
        z2s_16 = sb.tile([128, H, D_G], F16, tag="z2s")
        for h in range(H):
            nc.vector.tensor_mul(z2s_16[:, h, :], z2_16[:, h, :], isd16)

        if PHASE == "z4":
            y_z = sb.tile([128, D_G], F32, tag="yz")
            nc.vector.memset(y_z, 0.0)
            nc.sync.dma_start(
                out=y_out[r0 : r0 + 128, :].rearrange("(j s) g -> s j g", s=4),
                in_=y_z,
            )
            continue

        # ---- P5: shuffle to strip layout ----
        # z1a[32s+h, j, d] = z1[4j+s, h, d];  row 32s+8 = ones
        # z2a[32s+h, j, g] = z2s[4j+s, h, g]; row 32s+8 = -mu*isd
        # ---- P5: shuffle to per-chunk strip tiles (all at partition 0) ----
        # chunk ch = (s = ch//2, j0 = 16*(ch%2)): za_ch[h, jj, :] holds row
        # p = 32*s + j0 + jj; free dim = [z1 (256) | z2s (128)]; row 8 =
        # [ones | -mu*isd].
        blk_chunks = []
        for ch in range(8):
            s, j0 = ch // 2, 16 * (ch % 2)
            za_ch = strips.tile([9, 16, D_H + D_G], F16, tag="zch")
            p0 = 32 * s + j0
            for h in range(H):
                nc.sync.dma_start(
                    out=za_ch[h : h + 1, :, 0:D_H],
                    in_=z1_16[p0 : p0 + 16, h, :],
                )
                nc.sync.dma_start(
                    out=za_ch[h : h + 1, :, D_H : D_H + D_G],
                    in_=z2s_16[p0 : p0 + 16, h, :],
                )
            nc.sync.dma_start(
                out=za_ch[8:9, :, 0:D_H],
                in_=ones_h16[:, 0 : 16 * D_H].rearrange("p (j d) -> p j d", j=16),
            )
            nc.sync.dma_start(
                out=za_ch[8:9, :, D_H : D_H + D_G], in_=m2_16[p0 : p0 + 16, :]
            )
            blk_chunks.append((za_ch, s, j0))

        if PHASE == "z":
            y_zz = sb.tile([128, D_G], F32, tag="yz")
            nc.vector.memset(y_zz, 0.0)
            nc.sync.dma_start(
                out=y_out[r0 : r0 + 128, :].rearrange("(j s) g -> s j g", s=4),
                in_=y_zz,
            )
            continue

        # ---- P6: einsum-T (base-0 K=9 matmuls) + tanh + y ----
        for ch in range(8):
            za_ch, s, j0 = blk_chunks[ch]
            for rr in range(16):  # row within chunk
                p = 32 * s + j0 + rr          # permuted row within block
                half = (rr // 4) % 2
                q4 = rr % 4                    # row within ACT batch of 4
                for dh in range(2):
                    nc.tensor.matmul(
                        wt[:, 8 * half + 2 * q4 + dh, :],
                        lhsT=za_ch[0:9, rr, 128 * dh : 128 * (dh + 1)],
                        rhs=za_ch[0:9, rr, D_H : D_H + D_G],
                        start=True, stop=True,
                    )
                if q4 == 3:
                    tw = twp.tile([128, 8, 128], F16, tag="tw")
                    if PHASE == "ein":
                        nc.scalar.copy(tw, wt[:, 8 * half : 8 * half + 8, :])
                    else:
                        nc.scalar.activation(
                            tw, wt[:, 8 * half : 8 * half + 8, :], AF.Tanh
                        )
                    if PHASE in ("noy", "ein"):
                        continue
                    for q in range(4):  # rows p-3..p of this tanh batch
                        pq = p - 3 + q
                        for dh in range(2):
                            nc.tensor.matmul(
                                y_ps[:, 128 * blk + pq : 128 * blk + pq + 1],
                                lhsT=tw[:, 2 * q + dh, :],
                                rhs=hT16[:, dh, pq : pq + 1],
                                start=False, stop=(dh == 1),
                                skip_group_check=True,
                            )
        if PHASE in ("noy", "ein") and blk == NBLK - 1:
            y_zz = sb.tile([128, D_G], F32, tag="yz")
            nc.vector.memset(y_zz, 0.0)
            for b2 in range(NBLK):
                nc.sync.dma_start(
                    out=y_out[128 * b2 : 128 * b2 + 128, :].rearrange(
                        "(j s) g -> s j g", s=4
                    ),
                    in_=y_zz,
                )

    # ---- epilogue: transpose y (g, p) -> (p, g) and store ----
    if PHASE == "full":
        y_sbT = sb.tile([128, BLOC], F32, tag="ysbT")
        nc.vector.tensor_copy(y_sbT, y_ps)
        for b2 in range(NBLK):
            yt_ps = psZ.tile([128, 512], F32, tag="psZ")
            nc.tensor.transpose(
                yt_ps[:, 0:128], y_sbT[:, 128 * b2 : 128 * (b2 + 1)], ident32
            )
            y_fin = sb.tile([128, D_G], F32, tag="yfin")
            nc.scalar.copy(y_fin, yt_ps[:, 0:128])
            nc.sync.dma_start(
                out=y_out[128 * b2 : 128 * b2 + 128, :].rearrange(
                    "(j s) g -> s j g", s=4
                ),
                in_=y_fin,
            )


_CACHE = {}


def _get_program():
    if "prog" not in _CACHE:
        nc = bacc.Bacc("TRN2", target_bir_lowering=False, debug=False,
                       num_devices=NCORES)
        with tile.TileContext(nc) as tc:
            build(nc, tc)
        nc.compile()
        _CACHE["prog"] = nc
    return _CACHE["prog"]


def kernel(x, Wh, bh, Wz1, Wz2, ln_scale, ln_bias, _want_results=False):
    x = np.ascontiguousarray(np.asarray(x, dtype=np.float32))
    ins_common = {
        "Wh": np.ascontiguousarray(np.asarray(Wh, np.float32)),
        "bh": np.ascontiguousarray(np.asarray(bh, np.float32)),
        "Wz1": np.ascontiguousarray(np.asarray(Wz1, np.float32)),
        "Wz2": np.ascontiguousarray(np.asarray(Wz2, np.float32)),
        "ln_scale": np.ascontiguousarray(np.asarray(ln_scale, np.float32)),
        "ln_bias": np.ascontiguousarray(np.asarray(ln_bias, np.float32)),
    }
    in_maps = []
    for c in range(NCORES):
        m = dict(ins_common)
        m["x"] = np.ascontiguousarray(x[c * BLOC : (c + 1) * BLOC])
        in_maps.append(m)

    nc = _get_program()
    res = run_bass_kernel_spmd(nc, in_maps, core_ids=list(range(NCORES)))
    h = np.concatenate([r["h_out"] for r in res.results], axis=0)
    y = np.concatenate([r["y_out"] for r in res.results], axis=0)
    if _want_results:
        return (h, y), res
    return h, y


if __name__ == "__main__":
    rng = np.random.default_rng(0)
    ins = {
        "x": rng.standard_normal((B, D_IN), dtype=np.float32),
        "Wh": rng.standard_normal((D_IN, D_H), dtype=np.float32) / np.sqrt(D_IN),
        "bh": np.zeros((D_H,), np.float32),
        "Wz1": rng.standard_normal((D_IN, H * D_H), dtype=np.float32) / np.sqrt(D_IN),
        "Wz2": rng.standard_normal((D_IN, H * D_G), dtype=np.float32) / np.sqrt(D_IN),
        "ln_scale": np.ones((D_H,), np.float32),
        "ln_bias": np.zeros((D_H,), np.float32),
    }
    h, y = kernel(**ins)
    print(h.shape, y.shape)


# revision 67
# speedup vs baseline: 1.1055x; 1.1055x over previous
"""Trainium2 Bass kernel for nn_AttentionADCell.

Reference math (per batch row b, B=4096, D_IN=512, D_H=256, D_G=128, H=8):
  h  = LayerNorm(relu(x @ Wh + bh)) * ln_scale + ln_bias          (B, 256)
  z1 = (x @ Wz1).reshape(B, 8, 256)
  z2 = (x @ Wz2).reshape(B, 8, 128)
  w  = einsum('bhg,bhd->bgd', z2, z1)                              (B, 128, 256)
  w  = tanh((w - mean_d(w)) / (std_d(w) + 1e-6))
  y  = einsum('bgd,bd->bg', w, h)                                  (B, 128)
Returns (h, y).

Strategy: pure data parallel over batch across 8 NeuronCores (512 rows each).
Per core, rows are processed in 4 blocks of 128. Key ideas:
  - Standardization is folded into the einsum as a 9th "head":
      w' = (w - mu)*isd  ==  sum_h z1aug[h,d] * z2aug[h,g]
    with z1aug = [z1; ones], z2aug = [z2*isd; -mu*isd].  The einsum is done
    TRANSPOSED (d on partitions) so the final y contraction over d runs on
    the tensor engine.
  - mu and var are computed algebraically from small reductions:
      sum_d w = sum_h z2[h,g]*s1[h],  s1 = row-sums of z1
      sum_d w^2 = sum_{h,h'} z2[h]z2[h'] M1[h,h'],  M1 = z1 z1^T (8x8 Gram)
    M1 via 36 fused scalar_tensor_tensor+accum ops on the vector engine.
  - All einsum matmuls are K=9 at partition base 0 (tile_position packing is
    broken in this stack: zero in-repo usage, ignored by CoreSim, and any
    nonzero row position fails at runtime on HW).  y accumulates in a single
    PSUM bank in (g, row) orientation and is transposed once at the end.
  - fp16 for einsum operands / tanh output (fp32 accumulation in PSUM);
    the h path stays fp32 end-to-end.
"""

import os
import sys
for _p in ("/opt/trn_rl_repo", "/opt/pypackages"):
    if _p not in sys.path:
        sys.path.append(_p)

TP_ROW = os.environ.get("KT_NO_TPROW", "0") != "1"   # row-group packing
TP_COL = os.environ.get("KT_NO_TPCOL", "0") != "1"   # col-group packing
USE_DMAT = os.environ.get("KT_NO_DMAT", "0") != "1"  # hT via DMA-transpose
PHASE = os.environ.get("KT_PHASE", "full")  # h | z | noy | full

from contextlib import ExitStack

import numpy as np

import concourse.bass as bass
import concourse.bacc as bacc
import concourse.mybir as mybir
import concourse.tile as tile
from concourse.bass_utils import run_bass_kernel_spmd
from concourse.masks import make_identity

F32 = mybir.dt.float32
F32R = mybir.dt.float32r
F16 = mybir.dt.float16
AF = mybir.ActivationFunctionType
OP = mybir.AluOpType

D_IN, D_H, D_G, H, B = 512, 256, 128, 8, 4096
EPS_LN = 1e-6
EPS_STD = 1e-6
NCORES = 8
BLOC = B // NCORES          # rows per core = 512
NBLK = BLOC // 128          # 4 blocks of 128 rows

PAIRS = [(h, h) for h in range(H)] + [
    (h1, h2) for h1 in range(H) for h2 in range(h1 + 1, H)
]  # 8 diag + 28 off-diag = 36


def build(nc: bass.Bass, tc: tile.TileContext):
    x = nc.dram_tensor("x", [BLOC, D_IN], F32, kind="ExternalInput").ap()
    Wh = nc.dram_tensor("Wh", [D_IN, D_H], F32, kind="ExternalInput").ap()
    bh = nc.dram_tensor("bh", [D_H], F32, kind="ExternalInput").ap()
    Wz1 = nc.dram_tensor("Wz1", [D_IN, H * D_H], F32, kind="ExternalInput").ap()
    Wz2 = nc.dram_tensor("Wz2", [D_IN, H * D_G], F32, kind="ExternalInput").ap()
    ln_s = nc.dram_tensor("ln_scale", [D_H], F32, kind="ExternalInput").ap()
    ln_b = nc.dram_tensor("ln_bias", [D_H], F32, kind="ExternalInput").ap()
    h_out = nc.dram_tensor("h_out", [BLOC, D_H], F32, kind="ExternalOutput").ap()
    y_out = nc.dram_tensor("y_out", [BLOC, D_G], F32, kind="ExternalOutput").ap()

    with ExitStack() as ctx:
        _body(ctx, nc, tc, x, Wh, bh, Wz1, Wz2, ln_s, ln_b, h_out, y_out)


def _body(ctx, nc, tc, x, Wh, bh, Wz1, Wz2, ln_s, ln_b, h_out, y_out):
    consts = ctx.enter_context(tc.tile_pool(name="consts", bufs=1))
    sb = ctx.enter_context(tc.tile_pool(name="sb", bufs=2))
    strips = ctx.enter_context(tc.tile_pool(name="strips", bufs=4))
    stmp = ctx.enter_context(tc.tile_pool(name="stmp", bufs=1))
    small = ctx.enter_context(tc.tile_pool(name="small", bufs=2))
    scr = ctx.enter_context(tc.tile_pool(name="scr", bufs=4))
    twp = ctx.enter_context(tc.tile_pool(name="twp", bufs=3))
    psA = ctx.enter_context(tc.tile_pool(name="psA", bufs=1, space="PSUM"))
    psX = ctx.enter_context(tc.tile_pool(name="psX", bufs=1, space="PSUM"))
    psZ = ctx.enter_context(tc.tile_pool(name="psZ", bufs=1, space="PSUM"))
    psW = ctx.enter_context(tc.tile_pool(name="psW", bufs=1, space="PSUM"))
    psY = ctx.enter_context(tc.tile_pool(name="psY", bufs=1, space="PSUM"))

    # ---- constants ----
    ident32 = consts.tile([128, 128], F32)
    make_identity(nc, ident32)
    ones1_f32 = consts.tile([1, 128], F32)
    nc.vector.memset(ones1_f32, 1.0)
    onescol = consts.tile([128, 1], F32)
    nc.vector.memset(onescol, 1.0)
    zrow512 = consts.tile([1, 512], F32)
    nc.vector.memset(zrow512, 0.0)
    ones_h16 = consts.tile([1, 32 * D_H], F16)
    nc.vector.memset(ones_h16, 1.0)
    eps_ln = consts.tile([128, 1], F32)
    nc.vector.memset(eps_ln, EPS_LN)

    Wh_sb = consts.tile([128, 4, D_H], F32)
    nc.sync.dma_start(out=Wh_sb, in_=Wh.rearrange("(c p) n -> p c n", p=128))
    Wz1_sb = consts.tile([128, 4, H * D_H], F16)
    nc.gpsimd.dma_start(out=Wz1_sb, in_=Wz1.rearrange("(c p) n -> p c n", p=128))
    Wz2_sb = consts.tile([128, 4, H * D_G], F16)
    nc.gpsimd.dma_start(out=Wz2_sb, in_=Wz2.rearrange("(c p) n -> p c n", p=128))
    bh_sb = consts.tile([1, D_H], F32)
    nc.sync.dma_start(out=bh_sb, in_=bh.rearrange("(o n) -> o n", o=1))
    lns_sb = consts.tile([1, D_H], F32)
    nc.sync.dma_start(out=lns_sb, in_=ln_s.rearrange("(o n) -> o n", o=1))
    lnb_sb = consts.tile([1, D_H], F32)
    nc.sync.dma_start(out=lnb_sb, in_=ln_b.rearrange("(o n) -> o n", o=1))

    # broadcast ln scale/bias to (128, 256) via K=1 matmul
    lnsc_bc = consts.tile([128, D_H], F32)
    lnbi_bc = consts.tile([128, D_H], F32)
    bc_ps = psX.tile([128, D_H], F32, tag="psX")
    nc.tensor.matmul(bc_ps, lhsT=ones1_f32, rhs=lns_sb, start=True, stop=True)
    nc.scalar.copy(lnsc_bc, bc_ps)
    bc_ps2 = psX.tile([128, D_H], F32, tag="psX")
    nc.tensor.matmul(bc_ps2, lhsT=ones1_f32, rhs=lnb_sb, start=True, stop=True)
    nc.scalar.copy(lnbi_bc, bc_ps2)

    # wt: 4 PSUM banks, two 2-bank halves used alternately by einsum groups
    wt = psW.tile([128, 16, 128], F32)
    # y accumulator: one bank, col = permuted row index for the whole core
    y_ps = psY.tile([128, BLOC], F32)
    nc.tensor.matmul(y_ps, lhsT=ones1_f32, rhs=zrow512, start=True,
                     stop=False, skip_group_check=True)

    for blk in range(NBLK):
        r0 = blk * 128

        # ---- P1: load x (rows PERMUTED), transpose on PE ----
        # SBUF partition p = 32*s + j holds batch row r0 + 4*j + s: the strip
        # shuffle then reads contiguous partition ranges; output DMAs
        # un-permute.
        x_sb = sb.tile([128, D_IN], F32, tag="x")
        nc.sync.dma_start(
            out=x_sb, in_=x[r0 : r0 + 128, :].rearrange("(j s) d -> s j d", s=4)
        )
        xt_ps = psX.tile([128, D_IN], F32, tag="psX")
        for c in range(4):
            nc.tensor.transpose(
                xt_ps[:, 128 * c : 128 * (c + 1)],
                x_sb[:, 128 * c : 128 * (c + 1)],
                ident32,
            )
        xt_sb = sb.tile([128, 4, 128], F32, tag="xt")
        nc.scalar.copy(xt_sb, xt_ps.rearrange("p (c b) -> p c b", c=4))
        xt16 = sb.tile([128, 4, 128], F16, tag="xt16")
        nc.scalar.copy(xt16, xt_ps.rearrange("p (c b) -> p c b", c=4))

        # ---- P2: h path ----
        h_ps = psA.tile([128, D_H], F32, tag="psA")
        for c in range(4):
            nc.tensor.matmul(
                h_ps, lhsT=xt_sb[:, c, :], rhs=Wh_sb[:, c, :],
                start=(c == 0), stop=False,
            )
        nc.tensor.matmul(h_ps, lhsT=ones1_f32, rhs=bh_sb, start=False, stop=True)
        hr = sb.tile([128, D_H], F32, tag="hr")
        nc.scalar.activation(hr, h_ps, AF.Relu)
        st6 = small.tile([128, 6], F32, tag="st6")
        nc.vector.bn_stats(st6, hr)
        agg = small.tile([128, 2], F32, tag="agg")
        nc.vector.bn_aggr(agg, st6)
        sdt = small.tile([128, 1], F32, tag="sdt")
        nc.scalar.activation(sdt, agg[:, 1:2], AF.Sqrt, bias=eps_ln)
        rstd = small.tile([128, 1], F32, tag="rstd")
        nc.vector.reciprocal(rstd, sdt)
        nmr = small.tile([128, 1], F32, tag="nmr")
        nc.vector.scalar_tensor_tensor(
            out=nmr, in0=agg[:, 0:1], scalar=-1.0, in1=rstd,
            op0=OP.mult, op1=OP.mult,
        )
        hh = sb.tile([128, D_H], F32, tag="hh")
        nc.scalar.activation(hh, hr, AF.Identity, bias=nmr, scale=rstd)
        ht1 = sb.tile([128, D_H], F32, tag="ht1")
        nc.vector.tensor_mul(ht1, hh, lnsc_bc)
        h_sb = sb.tile([128, D_H], F32, tag="h")
        nc.vector.tensor_add(h_sb, ht1, lnbi_bc)
        nc.sync.dma_start(
            out=h_out[r0 : r0 + 128, :].rearrange("(j s) d -> s j d", s=4),
            in_=h_sb,
        )
        hT16 = sb.tile([128, 2, 128], F16, tag="hT")
        if USE_DMAT:
            h16 = sb.tile([128, D_H], F16, tag="h16")
            nc.scalar.copy(h16, h_sb)
            for t in range(2):
                nc.sync.dma_start_transpose(
                    out=hT16[:, t, :], in_=h16[:, 128 * t : 128 * (t + 1)]
                )
        else:
            ht_ps = psA.tile([128, D_H], F32, tag="psA")
            for t in range(2):
                nc.tensor.transpose(
                    ht_ps[:, 128 * t : 128 * (t + 1)],
                    h_sb[:, 128 * t : 128 * (t + 1)],
                    ident32,
                )
            nc.scalar.copy(hT16, ht_ps.rearrange("p (t b) -> p t b", t=2))

        if PHASE == "h":
            y_z = sb.tile([128, D_G], F32, tag="yz")
            nc.vector.memset(y_z, 0.0)
            nc.sync.dma_start(
                out=y_out[r0 : r0 + 128, :].rearrange("(j s) g -> s j g", s=4),
                in_=y_z,
            )
            continue

        # ---- P3: z projections (fp16), cast to fp16 ----
        z1_16 = sb.tile([128, H, D_H], F16, tag="z1b")
        z2_16 = sb.tile([128, H, D_G], F16, tag="z2b")
        for n in range(4):
            z_ps = psZ.tile([128, 512], F32, tag="psZ")
            for c in range(4):
                nc.tensor.matmul(
                    z_ps,
                    lhsT=xt16[:, c, :],
                    rhs=Wz1_sb[:, c, 512 * n : 512 * (n + 1)],
                    start=(c == 0), stop=(c == 3),
                )
            nc.vector.tensor_copy(
                z1_16[:, 2 * n : 2 * (n + 1), :],
                z_ps.rearrange("p (h d) -> p h d", h=2),
            )
        for n in range(2):
            z_ps = psZ.tile([128, 512], F32, tag="psZ")
            for c in range(4):
                nc.tensor.matmul(
                    z_ps,
                    lhsT=xt16[:, c, :],
                    rhs=Wz2_sb[:, c, 512 * n : 512 * (n + 1)],
                    start=(c == 0), stop=(c == 3),
                )
            nc.vector.tensor_copy(
                z2_16[:, 4 * n : 4 * (n + 1), :],
                z_ps.rearrange("p (h g) -> p h g", h=4),
            )
        s1 = small.tile([128, H], F32, tag="s1")
        nc.vector.tensor_reduce(s1, z1_16, axis=mybir.AxisListType.X, op=OP.add)

        if PHASE == "z3":
            y_z = sb.tile([128, D_G], F32, tag="yz")
            nc.vector.memset(y_z, 0.0)
            nc.sync.dma_start(
                out=y_out[r0 : r0 + 128, :].rearrange("(j s) g -> s j g", s=4),
                in_=y_z,
            )
            continue

        # ---- P4: stats (M1 Gram, mu, q, isd) ----
        M1 = small.tile([128, 36], F32, tag="M1")
        for idx, (h1, h2) in enumerate(PAIRS):
            scrt = scr.tile([128, D_H], F16, tag="scr")
            nc.vector.scalar_tensor_tensor(
                out=scrt, in0=z1_16[:, h1, :], scalar=1.0,
                in1=z1_16[:, h2, :], op0=OP.mult, op1=OP.mult,
                accum_out=M1[:, idx : idx + 1],
            )
        M1d = small.tile([128, 28], F32, tag="M1d")
        nc.vector.tensor_scalar_mul(M1d, M1[:, 8:36], 2.0)

        if PHASE == "m1":
            y_z = sb.tile([128, D_G], F32, tag="yz")
            nc.vector.memset(y_z, 0.0)
            nc.vector.tensor_scalar_mul(y_z[:, 0:1], M1[:, 0:1], 0.0)
            nc.vector.tensor_scalar_mul(y_z[:, 1:2], M1d[:, 0:1], 0.0)
            nc.sync.dma_start(
                out=y_out[r0 : r0 + 128, :].rearrange("(j s) g -> s j g", s=4),
                in_=y_z,
            )
            continue

        S16 = small.tile([128, D_G], F16, tag="S16")
        nc.vector.tensor_scalar(
            out=S16, in0=z2_16[:, 0, :], scalar1=s1[:, 0:1], scalar2=None,
            op0=OP.mult,
        )
        for h in range(1, H):
            nc.vector.scalar_tensor_tensor(
                out=S16, in0=z2_16[:, h, :], scalar=s1[:, h : h + 1], in1=S16,
                op0=OP.mult, op1=OP.add,
            )
        Q16 = small.tile([128, D_G], F16, tag="Q16")
        for idx, (h1, h2) in enumerate(PAIRS):
            prod = scr.tile([128, D_G], F16, tag="scr2")
            nc.vector.tensor_mul(prod, z2_16[:, h1, :], z2_16[:, h2, :])
            m1col = (
                M1[:, idx : idx + 1] if idx < 8 else M1d[:, idx - 8 : idx - 7]
            )
            if idx == 0:
                nc.vector.tensor_scalar(
                    out=Q16, in0=prod, scalar1=m1col, scalar2=None, op0=OP.mult
                )
            else:
                nc.vector.scalar_tensor_tensor(
                    out=Q16, in0=prod, scalar=m1col, in1=Q16,
                    op0=OP.mult, op1=OP.add,
                )

        if PHASE == "q":
            y_z = sb.tile([128, D_G], F32, tag="yz")
            nc.vector.tensor_scalar_mul(y_z, Q16, 0.0)
            nc.sync.dma_start(
                out=y_out[r0 : r0 + 128, :].rearrange("(j s) g -> s j g", s=4),
                in_=y_z,
            )
            continue

        mean = small.tile([128, D_G], F32, tag="mean")
        nc.vector.tensor_scalar_mul(mean, S16, 1.0 / D_H)
        msq = small.tile([128, D_G], F32, tag="msq")
        nc.vector.tensor_mul(msq, mean, mean)
        vt = small.tile([128, D_G], F32, tag="vt")
        nc.vector.tensor_scalar_mul(vt, Q16, 1.0 / D_H)
        nc.vector.tensor_sub(vt, vt, msq)
        nc.vector.tensor_scalar_max(vt, vt, 0.0)
        sd = small.tile([128, D_G], F32, tag="sd")
        nc.scalar.activation(sd, vt, AF.Sqrt)
        nc.vector.tensor_scalar_add(sd, sd, EPS_STD)
        isd = small.tile([128, D_G], F32, tag="isd")
        nc.vector.reciprocal(isd, sd)
        isd16 = small.tile([128, D_G], F16, tag="isd16")
        nc.vector.tensor_copy(isd16, isd)
        m2_16 = small.tile([128, D_G], F16, tag="m2")
        nc.vector.scalar_tensor_tensor(
            out=m2_16, in0=mean, scalar=-1.0, in1=isd, op0=OP.mult, op1=OP.mult
        )
        z2s_16 = sb.tile([128, H, D_G], F16, tag="z2s")
        for h in range(H):
            nc.vector.tensor_mul(z2s_16[:, h, :], z2_16[:, h, :], isd16)

        if PHASE == "z4":
            y_z = sb.tile([128, D_G], F32, tag="yz")
            nc.vector.memset(y_z, 0.0)
            nc.sync.dma_start(
                out=y_out[r0 : r0 + 128, :].rearrange("(j s) g -> s j g", s=4),
                in_=y_z,
            )
            continue

        # ---- P5: shuffle to strip layout ----
        # z1a[32s+h, j, d] = z1[4j+s, h, d];  row 32s+8 = ones
        # z2a[32s+h, j, g] = z2s[4j+s, h, g]; row 32s+8 = -mu*isd
        # ---- P5: shuffle to per-chunk strip tiles (all at partition 0) ----
        # chunk ch = (s = ch//2, j0 = 16*(ch%2)): za_ch[h, jj, :] holds row
        # p = 32*s + j0 + jj; free dim = [z1 (256) | z2s (128)]; row 8 =
        # [ones | -mu*isd].
        blk_chunks = []
        for ch in range(8):
            s, j0 = ch // 2, 16 * (ch % 2)
            za_ch = strips.tile([9, 16, D_H + D_G], F16, tag="zch")
            p0 = 32 * s + j0
            for h in range(H):
                nc.sync.dma_start(
                    out=za_ch[h : h + 1, :, 0:D_H],
                    in_=z1_16[p0 : p0 + 16, h, :],
                )
                nc.sync.dma_start(
                    out=za_ch[h : h + 1, :, D_H : D_H + D_G],
                    in_=z2s_16[p0 : p0 + 16, h, :],
                )
            nc.sync.dma_start(
                out=za_ch[8:9, :, 0:D_H],
                in_=ones_h16[:, 0 : 16 * D_H].rearrange("p (j d) -> p j d", j=16),
            )
            nc.sync.dma_start(
                out=za_ch[8:9, :, D_H : D_H + D_G], in_=m2_16[p0 : p0 + 16, :]
            )
            blk_chunks.append((za_ch, s, j0))

        if PHASE == "z":
            y_zz = sb.tile([128, D_G], F32, tag="yz")
            nc.vector.memset(y_zz, 0.0)
            nc.sync.dma_start(
                out=y_out[r0 : r0 + 128, :].rearrange("(j s) g -> s j g", s=4),
                in_=y_zz,
            )
            continue

        # ---- P6: einsum-T (base-0 K=9 matmuls) + tanh + y ----
        for ch in range(8):
            za_ch, s, j0 = blk_chunks[ch]
            for rr in range(16):  # row within chunk
                p = 32 * s + j0 + rr          # permuted row within block
                half = (rr // 4) % 2
                q4 = rr % 4                    # row within ACT batch of 4
                for dh in range(2):
                    nc.tensor.matmul(
                        wt[:, 8 * half + 2 * q4 + dh, :],
                        lhsT=za_ch[0:9, rr, 128 * dh : 128 * (dh + 1)],
                        rhs=za_ch[0:9, rr, D_H : D_H + D_G],
                        start=True, stop=True,
                    )
                if q4 == 3:
                    tw = twp.tile([128, 8, 128], F16, tag="tw")
                    if PHASE == "ein":
                        nc.scalar.copy(tw, wt[:, 8 * half : 8 * half + 8, :])
                    else:
                        nc.scalar.activation(
                            tw, wt[:, 8 * half : 8 * half + 8, :], AF.Tanh
                        )
                    if PHASE in ("noy", "ein"):
                        continue
                    for q in range(4):  # rows p-3..p of this tanh batch
                        pq = p - 3 + q
                        for dh in range(2):
                            nc.tensor.matmul(
                                y_ps[:, 128 * blk + pq : 128 * blk + pq + 1],
                                lhsT=tw[:, 2 * q + dh, :],
                                rhs=hT16[:, dh, pq : pq + 1],
                                start=False, stop=(dh == 1),
                                skip_group_check=True,
                            )
        if PHASE in ("noy", "ein") and blk == NBLK - 1:
            y_zz = sb.tile([128, D_G], F32, tag="yz")
            nc.vector.memset(y_zz, 0.0)
            for b2 in range(NBLK):
                nc.sync.dma_start(
                    out=y_out[128 * b2 : 128 * b2 + 128, :].rearrange(
                        "(j s) g -> s j g", s=4
                    ),
                    in_=y_zz,
                )

    # ---- epilogue: transpose y (g, p) -> (p, g) and store ----
    if PHASE == "full":
        y_sbT = sb.tile([128, BLOC], F32, tag="ysbT")
        nc.vector.tensor_copy(y_sbT, y_ps)
        for b2 in range(NBLK):
            yt_ps = psZ.tile([128, 512], F32, tag="psZ")
            nc.tensor.transpose(
                yt_ps[:, 0:128], y_sbT[:, 128 * b2 : 128 * (b2 + 1)], ident32
            )
            y_fin = sb.tile([128, D_G], F32, tag="yfin")
            nc.scalar.copy(y_fin, yt_ps[:, 0:128])
            nc.sync.dma_start(
                out=y_out[128 * b2 : 128 * b2 + 128, :].rearrange(
                    "(j s) g -> s j g", s=4
                ),
                in_=y_fin,
            )


_CACHE = {}


def _get_program():
    if "prog" not in _CACHE:
        nc = bacc.Bacc("TRN2", target_bir_lowering=False, debug=False,
                       num_devices=NCORES)
        with tile.TileContext(nc) as tc:
            build(nc, tc)
        nc.compile()
        _CACHE["prog"] = nc
    return _CACHE["prog"]


def kernel(x, Wh, bh, Wz1, Wz2, ln_scale, ln_bias, _want_results=False):
    x = np.ascontiguousarray(np.asarray(x, dtype=np.float32))
    ins_common = {
        "Wh": np.ascontiguousarray(np.asarray(Wh, np.float32)),
        "bh": np.ascontiguousarray(np.asarray(bh, np.float32)),
        "Wz1": np.ascontiguousarray(np.asarray(Wz1, np.float32)),
        "Wz2": np.ascontiguousarray(np.asarray(Wz2, np.float32)),
        "ln_scale": np.ascontiguousarray(np.asarray(ln_scale, np.float32)),
        "ln_bias": np.ascontiguousarray(np.asarray(ln_bias, np.float32)),
    }
    in_maps = []
    for c in range(NCORES):
        m = dict(ins_common)
        m["x"] = np.ascontiguousarray(x[c * BLOC : (c + 1) * BLOC])
        in_maps.append(m)

    nc = _get_program()
    res = run_bass_kernel_spmd(nc, in_maps, core_ids=list(range(NCORES)))
    h = np.concatenate([r["h_out"] for r in res.results], axis=0)
    y = np.concatenate([r["y_out"] for r in res.results], axis=0)
    if _want_results:
        return (h, y), res
    return h, y


if __name__ == "__main__":
    rng = np.random.default_rng(0)
    ins = {
        "x": rng.standard_normal((B, D_IN), dtype=np.float32),
        "Wh": rng.standard_normal((D_IN, D_H), dtype=np.float32) / np.sqrt(D_IN),
        "bh": np.zeros((D_H,), np.float32),
        "Wz1": rng.standard_normal((D_IN, H * D_H), dtype=np.float32) / np.sqrt(D_IN),
        "Wz2": rng.standard_normal((D_IN, H * D_G), dtype=np.float32) / np.sqrt(D_IN),
        "ln_scale": np.ones((D_H,), np.float32),
        "ln_bias": np.zeros((D_H,), np.float32),
    }
    h, y = kernel(**ins)
    print(h.shape, y.shape)
